# revision 9
# baseline (speedup 1.0000x reference)
"""Trainium2 Bass kernel for nn_AttentionModel (pointer-network decode step).

Data-parallel over 8 NeuronCores: batch 512 -> 64 samples/core; weights
replicated.  v3: all streamed K/V slabs and weights are fp16 (~85 MB/core of
HBM reads) with host-side layouts interleaved per 4-sample group so every DMA
is one ~1 MB transfer with 8 KB-contiguous per-partition descriptors.  Stream
tile pools are shared across the three attention phases so the DMA queues
never drain at phase boundaries; attention group loops are software-pipelined
(scores of group g overlap softmax+values of group g-1).

Per core the network is:
  self-attn over (K_sa | k_sa) -> LN -> enc attention (masked) -> LN ->
  MLP -> LN -> single-head tanh-clipped pointer scores -> softmax weights.

Layouts (per core, 64 samples):
  - activations kept batch-major f32 [64, 512] (layernorm/residual) and
    d-major fp16 [128, 4, 64] (matmul operands).
  - K streams host-transposed d-major per group [g, 128, 4j, 4c, 256] fp16;
    scores accumulate over 4 d-chunks with keys as the moving dim.  V streams
    key-major [g, 128, 4j, 2c, 512] fp16.
  - per-sample multi-head scores live in PSUM as 16-head bands at 32-row
    offsets (4 samples per [128, n] tile, PE column-group packing); softmax on
    whole tiles.  esc -> PE transpose (fp16) -> value matmuls; the appended
    self-attn token is handled by a per-group rank-4 matmul (E4 transpose of
    the esc extra column x the 4 new-token value rows).
"""

import numpy as np
from contextlib import ExitStack

import concourse.bass as bass
import concourse.tile as tile
from concourse import bacc, mybir
from concourse.bass_utils import run_bass_kernel_spmd

f32 = mybir.dt.float32
f16 = mybir.dt.float16
AF = mybir.ActivationFunctionType
ALU = mybir.AluOpType
AX = mybir.AxisListType

P = 128          # SBUF partitions
NCORES = 8
B = 512          # full batch
BC = B // NCORES # batch per core (64)
D = 512          # model dim
H = 16           # heads
DH = 32          # head dim
NK = 251         # encoder keys (nb_nodes + 1)
NP = 256         # encoder keys padded to 256
T = 256          # self-attn cache length (new token appended on device)
NG = BC // 4     # sample groups of 4 (one [128, n] psum tile each)

WNAMES = ["Wq_sa", "Wk_sa", "Wv_sa", "W0_sa", "Wq_a", "W0_a", "W1", "W2", "Wqf"]
EARLY_W = ["Wq_sa", "Wk_sa", "Wv_sa"]   # needed before layer-1 attention
# weight matmuls whose bias is applied on batch-major [64, 512] rows
BM_BIAS = {"Wv_sa", "W0_sa", "W0_a", "W2"}

_cache = {}


def _expandcol(ap3, n):
    """Replace a trailing size-1 dim with a 0-stride dim of size n."""
    a = [list(x) for x in ap3.ap]
    assert a[-1][1] == 1
    a[-1] = [0, n]
    return bass.AP(tensor=ap3.tensor, offset=ap3.offset, ap=a)


def _bcast_row(ap2d, i, n):
    row = ap2d[i:i + 1, :]
    return bass.AP(tensor=row.tensor, offset=row.offset,
                   ap=[[0, n]] + list(row.ap)[1:])


# ----------------------------------------------------------------------------
# program builder
# ----------------------------------------------------------------------------

def _build_program(flags):
    """flags = (use_bias tuple aligned with WNAMES, ln_affine tuple of 3)."""
    use_bias = dict(zip(WNAMES, flags[0]))
    ln_affine = flags[1]

    nc = bacc.Bacc("TRN2", target_bir_lowering=False, debug=False)

    def din(name, shape, dt=f32):
        return nc.dram_tensor(name, shape, dt, kind="ExternalInput").ap()

    hT_d = din("hT", [P, 4, BC], f16)
    hbm_d = din("h_bm", [BC, D])
    ksa_d = din("ksa", [NG, P, 4, 4, NP], f16)
    vsa_d = din("vsa", [NG, P, 4, 2, D], f16)
    ka0_d = din("ka0", [NG, P, 4, 4, NP], f16)
    va0_d = din("va0", [NG, P, 4, 2, D], f16)
    kaf_d = din("kaf", [NG, P, 4, 4, NP], f16)
    mask2_d = din("mask2", [NG, P, NP], f16)
    W_d = {n: din("W_" + n, [P, 4, D], f16) for n in WNAMES}
    b_d = {n: din("b_" + n, [P, 4]) for n in WNAMES if use_bias[n]}
    bf_d = {n: din("bf_" + n, [1, D]) for n in WNAMES
            if use_bias[n] and n in BM_BIAS}
    if any(ln_affine):
        lnp_d = din("lnp", [6, D])
    ident_d = din("ident", [P, P], f16)
    ones4_d = din("ones4", [P, 4], f16)
    blk4_d = din("blk4", [P, 4], f16)
    bm4_d = din("bm4", [P, D])
    qm_d = din("qm", [P, 4, DH], f16)
    vscr_d = nc.dram_tensor("vscr", [BC, D], f16, kind="Internal").ap()

    out_d = nc.dram_tensor("out", [BC, NK], f32, kind="ExternalOutput").ap()

    with tile.TileContext(nc) as tc, ExitStack() as ctx:
        consts = ctx.enter_context(tc.tile_pool(name="consts", bufs=1))
        acts = ctx.enter_context(tc.tile_pool(name="acts", bufs=1))
        small = ctx.enter_context(tc.tile_pool(name="small", bufs=24))
        big_tmp = ctx.enter_context(tc.tile_pool(name="big_tmp", bufs=2))
        # stream pools shared by all three attention phases; deep enough that
        # DMA keeps prefetching through the serial x-compute at phase edges
        pk = ctx.enter_context(tc.tile_pool(name="pk", bufs=6))
        pv = ctx.enter_context(tc.tile_pool(name="pv", bufs=7))

        # ------------------------------------------------------------------
        # constants / early weights (late weights stream during layer 1)
        # ------------------------------------------------------------------
        qm = consts.tile([P, 4, DH], f16, name="qm", tag="qm")
        nc.scalar.dma_start(out=qm, in_=qm_d)
        ident = consts.tile([P, P], f16, name="ident", tag="ident")
        nc.scalar.dma_start(out=ident, in_=ident_d)
        ones4 = consts.tile([P, 4], f16, name="ones4", tag="ones4")
        nc.scalar.dma_start(out=ones4, in_=ones4_d)
        blk4 = consts.tile([P, 4], f16, name="blk4", tag="blk4")
        nc.scalar.dma_start(out=blk4, in_=blk4_d)
        bm4 = consts.tile([P, D], f32, name="bm4", tag="bm4")
        nc.scalar.dma_start(out=bm4, in_=bm4_d)
        eps = consts.tile([P, 1], f32, name="eps", tag="eps")
        nc.vector.memset(eps, 1e-5)

        x0T = acts.tile([P, 4, BC], f16, name="x0T", tag="x0T")
        nc.scalar.dma_start(out=x0T, in_=hT_d)
        h_bm = acts.tile([BC, D], f32, name="h_bm", tag="h_bm")
        nc.scalar.dma_start(out=h_bm, in_=hbm_d)

        mask_all = consts.tile([P, NG, NP], f16, name="mask_all", tag="mask_all")
        nc.gpsimd.dma_start(
            out=mask_all, in_=mask2_d.rearrange("g p n -> p g n"))

        Wt, bt, bft = {}, {}, {}
        for n in WNAMES:
            Wt[n] = consts.tile([P, 4, D], f16, name="W_" + n, tag="W_" + n)
            if use_bias[n]:
                bt[n] = consts.tile([P, 4], f32, name="b_" + n, tag="b_" + n)
                if n in BM_BIAS:
                    bft[n] = consts.tile([BC, D], f32, name="bf_" + n,
                                         tag="bf_" + n)

        def load_weight(n, engine):
            engine.dma_start(out=Wt[n], in_=W_d[n])
            if use_bias[n]:
                nc.scalar.dma_start(out=bt[n], in_=b_d[n])
                if n in BM_BIAS:
                    nc.scalar.dma_start(out=bft[n], in_=_bcast_row(bf_d[n], 0, BC))

        for n in EARLY_W:
            load_weight(n, nc.sync)

        lng, lnb = [None] * 3, [None] * 3
        for i in range(3):
            if ln_affine[i]:
                lng[i] = consts.tile([BC, D], f32, name=f"lng{i}", tag=f"lng{i}")
                nc.scalar.dma_start(out=lng[i], in_=_bcast_row(lnp_d, 2 * i, BC))
                lnb[i] = consts.tile([BC, D], f32, name=f"lnb{i}", tag=f"lnb{i}")
                nc.scalar.dma_start(out=lnb[i], in_=_bcast_row(lnp_d, 2 * i + 1, BC))

        # ------------------------------------------------------------------
        # helpers
        # ------------------------------------------------------------------
        def proj_dmajor(dst, wname, src_T, relu=False):
            """dst[:, mc, :] (d-major fp16 [128, 4, 64]) = act(x @ W + b)."""
            with tc.tile_pool(name="pp_" + wname, bufs=2, space="PSUM") as pp:
                for mc in range(4):
                    ps = pp.tile([P, BC], f32, name="ps", tag="ps")
                    for kc in range(4):
                        nc.tensor.matmul(
                            ps,
                            lhsT=Wt[wname][:, kc, mc * P:(mc + 1) * P],
                            rhs=src_T[:, kc, :],
                            start=(kc == 0), stop=(kc == 3),
                        )
                    bias = bt[wname][:, mc:mc + 1] if use_bias[wname] else 0.0
                    func = AF.Relu if relu else AF.Identity
                    nc.scalar.activation(dst[:, mc, :], ps, func, bias=bias,
                                         scale=1.0)

        def mm_batchmajor(ps, src_T, wname):
            """ps [64, 512] f32 = x @ W   (lhsT = x^T chunks, W moving)."""
            for kc in range(4):
                nc.tensor.matmul(
                    ps,
                    lhsT=src_T[:, kc, :],
                    rhs=Wt[wname][:, kc, :],
                    start=(kc == 0), stop=(kc == 3),
                )

        def to_dmajor(dst_T, src_bm16):
            """[64, 512] fp16 batch-major -> d-major fp16 [128, 4, 64]."""
            with tc.tile_pool(name="ptr", bufs=2, space="PSUM") as pt:
                for c in range(4):
                    ps = pt.tile([P, BC], f16, name="ps", tag="ps")
                    nc.tensor.transpose(ps, src_bm16[:, c * P:(c + 1) * P],
                                        ident[0:BC, 0:BC])
                    nc.scalar.copy(dst_T[:, c, :], ps)

        def layer_norm(dst_bm, t_bm, idx):
            stats = small.tile([BC, 6], f32, name="stats", tag="stats")
            nc.vector.bn_stats(stats, t_bm)
            mv = small.tile([BC, 2], f32, name="mv", tag="mv")
            nc.vector.bn_aggr(mv, stats)
            sd = small.tile([BC, 1], f32, name="sd", tag="sd")
            nc.scalar.activation(sd, mv[:, 1:2], AF.Sqrt, bias=eps[0:BC],
                                 scale=1.0)
            rstd = small.tile([BC, 1], f32, name="rstd", tag="rstd")
            nc.vector.reciprocal(rstd, sd)
            nmr = small.tile([BC, 1], f32, name="nmr", tag="nmr")
            nc.vector.scalar_tensor_tensor(out=nmr, in0=mv[:, 0:1], scalar=-1.0,
                                           in1=rstd, op0=ALU.mult, op1=ALU.mult)
            if ln_affine[idx]:
                xn = big_tmp.tile([BC, D], f32, name="xn", tag="xn")
                nc.scalar.activation(xn, t_bm, AF.Identity, bias=nmr, scale=rstd)
                nc.vector.tensor_mul(xn, xn, lng[idx])
                nc.vector.tensor_add(dst_bm, xn, lnb[idx])
            else:
                nc.scalar.activation(dst_bm, t_bm, AF.Identity, bias=nmr,
                                     scale=rstd)

        def residual_ln(dst_bm, dst_T, src_T, wname, x_prev_bm, idx):
            """dst = LN(x_prev + src @ W + b); also d-major fp16 dst_T."""
            with tc.tile_pool(name="pr_" + wname, bufs=1, space="PSUM") as pr:
                ps = pr.tile([BC, D], f32, name="ps", tag="ps")
                mm_batchmajor(ps, src_T, wname)
                t_bm = big_tmp.tile([BC, D], f32, name="t_bm", tag="t_bm")
                nc.vector.tensor_add(t_bm, ps, x_prev_bm)
                if use_bias[wname]:
                    nc.vector.tensor_add(t_bm, t_bm, bft[wname])
                layer_norm(dst_bm, t_bm, idx)
            bm16 = big_tmp.tile([BC, D], f16, name="bm16", tag="bm16")
            nc.vector.tensor_copy(bm16, dst_bm)
            to_dmajor(dst_T, bm16)

        # ------------------------------------------------------------------
        # projections from x0 = h_t
        # ------------------------------------------------------------------
        q_saT = acts.tile([P, 4, BC], f16, name="q_saT", tag="q_saT")
        proj_dmajor(q_saT, "Wq_sa", x0T)
        k_saT = acts.tile([P, 4, BC], f16, name="k_saT", tag="k_saT")
        proj_dmajor(k_saT, "Wk_sa", x0T)

        # v = x0 @ Wv_sa, fp16 batch-major, then regrouped [4, 16, 512]
        # (vr4[j, g, :] = v[4g + j]) for the per-group rank-4 extra-value mm.
        v16 = acts.tile([BC, D], f16, name="v16", tag="v16")
        with tc.tile_pool(name="pv0", bufs=1, space="PSUM") as pv0:
            psv = pv0.tile([BC, D], f32, name="psv", tag="psv")
            mm_batchmajor(psv, x0T, "Wv_sa")
            if use_bias["Wv_sa"]:
                vtmp = big_tmp.tile([BC, D], f32, name="vtmp", tag="vtmp")
                nc.vector.tensor_add(vtmp, psv, bft["Wv_sa"])
                nc.vector.tensor_copy(v16, vtmp)
            else:
                nc.vector.tensor_copy(v16, psv)
        vr4 = acts.tile([4, NG, D], f16, name="vr4", tag="vr4")
        nc.scalar.dma_start(out=vscr_d, in_=v16)
        vsr = vscr_d.rearrange("(g j) d -> j g d", j=4)
        for j in range(4):
            nc.scalar.dma_start(out=vr4[j:j + 1, :, :], in_=vsr[j:j + 1])

        # ------------------------------------------------------------------
        # attention layers (software-pipelined over 16 groups of 4 samples)
        # ------------------------------------------------------------------
        def attention(qT, kstream_d, vstream_d, attn16, extra, masked,
                      after_scores0=None):
            """attn16 [64, 512] fp16 <- per-sample MHA.

            extra: include the appended self-attn token (k from k_saT, v from
            vr4).  masked: add the -30000 encoder mask tiles before softmax.
            after_scores0: callback emitted after group 0's scores (used to
            stream the late weights without delaying the first attention DMA).
            """
            ncol = NP + 1 if extra else NP
            with (
                tc.tile_pool(name="at_qbd", bufs=2) as pqbd,
                tc.tile_pool(name="at_esc", bufs=2) as pesc,
                tc.tile_pool(name="at_wt", bufs=2) as pwt,
                tc.tile_pool(name="at_e4", bufs=2) as pe4,
                tc.tile_pool(name="at_w256", bufs=2) as pw256,
                tc.tile_pool(name="at_ex", bufs=2) as pex,
                tc.tile_pool(name="at_a4", bufs=2) as pa4,
                tc.tile_pool(name="at_psc", bufs=2, space="PSUM") as psc,
                tc.tile_pool(name="at_pswt", bufs=2, space="PSUM") as pswt,
                tc.tile_pool(name="at_pspt", bufs=2, space="PSUM") as pspt,
                tc.tile_pool(name="at_psa4", bufs=1, space="PSUM") as psa4,
            ):
                def stage_scores(g):
                    kt = pk.tile([P, 4, 4, NP], f16, name="kt", tag="kt")
                    nc.sync.dma_start(out=kt, in_=kstream_d[g])
                    vt = pv.tile([P, 4, 2, D], f16, name="vt", tag="vt")
                    nc.sync.dma_start(out=vt, in_=vstream_d[g])
                    qbd = pqbd.tile([P, 4, 4, DH], f16, name="qbd", tag="qbd")
                    for j in range(4):
                        b = 4 * g + j
                        nc.vector.tensor_mul(
                            qbd[:, j, :, :], qm,
                            _expandcol(qT[:, :, b:b + 1], DH))
                    ps_sc = psc.tile([P, ncol], f32, name="ps_sc", tag="ps_sc")
                    for j in range(4):
                        b = 4 * g + j
                        for c in range(4):
                            nc.tensor.matmul(
                                ps_sc[32 * j:32 * j + 32, 0:NP],
                                lhsT=qbd[:, j, c, :],
                                rhs=kt[:, j, c, :],
                                start=(c == 0), stop=(False if extra else c == 3),
                                tile_position=(0, 32 * j))
                            if extra:
                                nc.tensor.matmul(
                                    ps_sc[32 * j:32 * j + 32, NP:NP + 1],
                                    lhsT=qbd[:, j, c, :],
                                    rhs=k_saT[:, c, b:b + 1],
                                    start=False, stop=(c == 3),
                                    tile_position=(0, 32 * j))
                    return ps_sc, vt

                def stage_rest(g, ps_sc, vt):
                    if masked:
                        nc.vector.tensor_add(ps_sc[:, 0:NP], ps_sc[:, 0:NP],
                                             mask_all[:, g, :])
                    negmax = small.tile([P, 1], f32, name="negmax", tag="negmax")
                    nc.vector.tensor_reduce(negmax, ps_sc, axis=AX.X,
                                            op=ALU.max, negate=True)
                    esc = pesc.tile([P, ncol], f16, name="esc", tag="esc")
                    sumexp = small.tile([P, 1], f32, name="sumexp", tag="sumexp")
                    nc.scalar.activation(esc, ps_sc, AF.Exp, bias=negmax,
                                         scale=1.0, accum_out=sumexp)
                    recip = small.tile([P, 1], f32, name="recip", tag="recip")
                    nc.vector.reciprocal(recip, sumexp)

                    ps_wt = pswt.tile([P, 3, P], f16, name="ps_wt", tag="ps_wt")
                    for c in range(2):
                        nc.tensor.transpose(ps_wt[:, c, :],
                                            esc[:, c * P:(c + 1) * P], ident)
                    wt = pwt.tile([P, 2, P], f16, name="wt", tag="wt")
                    for c in range(2):
                        nc.scalar.copy(wt[:, c, :], ps_wt[:, c, :])

                    if extra:
                        e4 = pe4.tile([P, 4], f16, name="e4", tag="e4")
                        nc.vector.tensor_mul(
                            e4, blk4, _expandcol(esc[:, NP:NP + 1], 4))
                        nc.tensor.transpose(ps_wt[0:4, 2, :], e4, ident)
                        w256 = pw256.tile([4, P], f16, name="w256", tag="w256")
                        nc.scalar.copy(w256, ps_wt[0:4, 2, :])

                    ps_pt = pspt.tile([P, D], f32, name="ps_pt", tag="ps_pt")
                    for j in range(4):
                        for kc in range(2):
                            nc.tensor.matmul(
                                ps_pt[32 * j:32 * j + 32, :],
                                lhsT=wt[:, kc, 32 * j:32 * j + 32],
                                rhs=vt[:, j, kc, :],
                                start=(kc == 0), stop=(kc == 1),
                                tile_position=(0, 32 * j))
                    if extra:
                        nc.tensor.matmul(
                            ps_pt, lhsT=w256, rhs=vr4[:, g, :],
                            start=False, stop=True, skip_group_check=True)
                    ex = pex.tile([P, D], f16, name="ex", tag="ex")
                    nc.vector.scalar_tensor_tensor(
                        out=ex, in0=ps_pt, scalar=recip, in1=bm4,
                        op0=ALU.mult, op1=ALU.mult)
                    ps_a4 = psa4.tile([4, D], f32, name="ps_a4", tag="ps_a4")
                    nc.tensor.matmul(ps_a4, lhsT=ones4, rhs=ex,
                                     start=True, stop=True)
                    a4 = pa4.tile([4, D], f16, name="a4", tag="a4")
                    nc.vector.tensor_copy(a4, ps_a4)
                    nc.scalar.dma_start(out=attn16[4 * g:4 * g + 4, :], in_=a4)

                state = {}
                for g in range(NG + 1):
                    if g < NG:
                        state[g] = stage_scores(g)
                        if g == 0 and after_scores0 is not None:
                            after_scores0()
                    if g > 0:
                        stage_rest(g - 1, *state.pop(g - 1))

        # ------------------------------------------------------------------
        # layer 1: self-attention over (K_sa | k_sa)
        # ------------------------------------------------------------------
        def load_late_weights():
            for n in WNAMES:
                if n not in EARLY_W:
                    load_weight(n, nc.gpsimd)

        attn1_16 = acts.tile([BC, D], f16, name="attn1_16", tag="attn1_16")
        attention(q_saT, ksa_d, vsa_d, attn1_16, extra=True, masked=False,
                  after_scores0=load_late_weights)

        attn1T = acts.tile([P, 4, BC], f16, name="attn1T", tag="attn1T")
        to_dmajor(attn1T, attn1_16)
        x1_bm = acts.tile([BC, D], f32, name="x1_bm", tag="x1_bm")
        x1T = acts.tile([P, 4, BC], f16, name="x1T", tag="x1T")
        residual_ln(x1_bm, x1T, attn1T, "W0_sa", h_bm, 0)

        # ------------------------------------------------------------------
        # layer 2: encoder attention (masked, padded keys)
        # ------------------------------------------------------------------
        q_aT = acts.tile([P, 4, BC], f16, name="q_aT", tag="q_aT")
        proj_dmajor(q_aT, "Wq_a", x1T)
        attn2_16 = acts.tile([BC, D], f16, name="attn2_16", tag="attn2_16")
        attention(q_aT, ka0_d, va0_d, attn2_16, extra=False, masked=True)

        attn2T = acts.tile([P, 4, BC], f16, name="attn2T", tag="attn2T")
        to_dmajor(attn2T, attn2_16)
        x2_bm = acts.tile([BC, D], f32, name="x2_bm", tag="x2_bm")
        x2T = acts.tile([P, 4, BC], f16, name="x2T", tag="x2T")
        residual_ln(x2_bm, x2T, attn2T, "W0_a", x1_bm, 1)

        # ------------------------------------------------------------------
        # MLP
        # ------------------------------------------------------------------
        h1T = acts.tile([P, 4, BC], f16, name="h1T", tag="h1T")
        proj_dmajor(h1T, "W1", x2T, relu=True)
        x3_bm = acts.tile([BC, D], f32, name="x3_bm", tag="x3_bm")
        x3T = acts.tile([P, 4, BC], f16, name="x3T", tag="x3T")
        residual_ln(x3_bm, x3T, h1T, "W2", x2_bm, 2)

        qfT = acts.tile([P, 4, BC], f16, name="qfT", tag="qfT")
        proj_dmajor(qfT, "Wqf", x3T)

        # ------------------------------------------------------------------
        # final pointer scores: w = softmax(10*tanh(qf.K/sqrt(D)) + mask)
        # ------------------------------------------------------------------
        with (
            tc.tile_pool(name="ft", bufs=2) as pft,
            tc.tile_pool(name="fe", bufs=2) as pfe,
            tc.tile_pool(name="fw", bufs=2) as pfw,
            tc.tile_pool(name="fps", bufs=2, space="PSUM") as psf,
        ):
            def f_scores(g):
                kt = pk.tile([P, 4, 4, NP], f16, name="kt", tag="kt")
                nc.sync.dma_start(out=kt, in_=kaf_d[g])
                ps_f = psf.tile([P, NP], f32, name="ps_f", tag="ps_f")
                nc.vector.memset(ps_f, 0.0)
                for j in range(4):
                    b = 4 * g + j
                    for c in range(4):
                        nc.tensor.matmul(
                            ps_f[32 * j:32 * j + 1, :],
                            lhsT=qfT[:, c, b:b + 1],
                            rhs=kt[:, j, c, :],
                            start=(c == 0), stop=(c == 3),
                            tile_position=(0, 32 * j))
                return (ps_f,)

            def f_rest(g, ps_f):
                t1 = pft.tile([P, NP], f32, name="t1", tag="t1")
                nc.scalar.activation(t1, ps_f, AF.Tanh, scale=float(D) ** -0.5)
                t2 = pft.tile([P, NP], f32, name="t2", tag="t2")
                nc.vector.scalar_tensor_tensor(out=t2, in0=t1, scalar=10.0,
                                               in1=mask_all[:, g, :],
                                               op0=ALU.mult, op1=ALU.add)
                e = pfe.tile([P, NP], f32, name="e", tag="e")
                sumexp = small.tile([P, 1], f32, name="fsum", tag="fsum")
                nc.scalar.activation(e, t2, AF.Exp, accum_out=sumexp)
                recip = small.tile([P, 1], f32, name="frec", tag="frec")
                nc.vector.reciprocal(recip, sumexp)
                wf = pfw.tile([P, NK], f32, name="wf", tag="wf")
                nc.vector.tensor_scalar_mul(wf, e[:, 0:NK], recip)
                nc.scalar.dma_start(
                    out=out_d[4 * g:4 * g + 4, :],
                    in_=wf.rearrange("(a b) n -> a b n", b=32)[:, 0, :])

            state = {}
            for g in range(NG + 1):
                if g < NG:
                    state[g] = f_scores(g)
                if g > 0:
                    f_rest(g - 1, *state.pop(g - 1))

    nc.compile()
    return nc


# ----------------------------------------------------------------------------
# host side
# ----------------------------------------------------------------------------

def _get_program(flags):
    if flags not in _cache:
        _cache[flags] = _build_program(flags)
    return _cache[flags]


def _prep_inputs(inputs):
    """Host-side sharding + layout prep; returns (flags, per-core inputs)."""
    f = np.float32
    h = np.float16
    h_t = np.asarray(inputs["h_t"], f)
    K_att = np.asarray(inputs["K_att"], f)
    V_att = np.asarray(inputs["V_att"], f)
    K_sa = np.asarray(inputs["K_sa"], f)
    V_sa = np.asarray(inputs["V_sa"], f)
    mask = np.asarray(inputs["mask"])

    sc = np.float32(DH ** -0.5)
    W = {n: np.asarray(inputs[n], f) for n in WNAMES}
    W["Wq_sa"] = W["Wq_sa"] * sc
    W["Wq_a"] = W["Wq_a"] * sc
    bias_src = {"Wq_sa": "bq_sa", "Wk_sa": "bk_sa", "Wv_sa": "bv_sa",
                "W0_sa": "b0_sa", "Wq_a": "bq_a", "W0_a": "b0_a",
                "W1": "b1", "W2": "b2", "Wqf": "bqf"}
    bvec = {n: np.asarray(inputs[bias_src[n]], f).copy() for n in WNAMES}
    bvec["Wq_sa"] *= sc
    bvec["Wq_a"] *= sc
    use_bias = tuple(bool(np.any(bvec[n])) for n in WNAMES)
    ub = dict(zip(WNAMES, use_bias))

    lnp = np.stack([np.asarray(inputs[k], f) for k in
                    ["ln1_g", "ln1_b", "ln2_g", "ln2_b", "ln3_g", "ln3_b"]])
    ln_affine = tuple(
        bool(np.any(lnp[2 * i] != 1.0) or np.any(lnp[2 * i + 1] != 0.0))
        for i in range(3))
    flags = (use_bias, ln_affine)

    # d-major group-interleaved K streams: [NCORES, NG, 128, 4j, 4c, 256] fp16
    def kfat(Ksl):
        kT = Ksl.transpose(0, 2, 1)                     # (B, D, n)
        if kT.shape[2] != NP:
            kp = np.zeros((B, D, NP), f)
            kp[:, :, :kT.shape[2]] = kT
            kT = kp
        a = kT.reshape(NCORES, NG, 4, 4, P, NP)         # (i, g, j, c, p, k)
        return np.ascontiguousarray(
            a.transpose(0, 1, 4, 2, 3, 5).astype(h))    # (i, g, p, j, c, k)

    # key-major group-interleaved V streams: [NCORES, NG, 128, 4j, 2c, 512]
    def vfat(Vsl):
        if Vsl.shape[1] != NP:
            vp = np.zeros((B, NP, D), f)
            vp[:, :Vsl.shape[1], :] = Vsl
            Vsl = vp
        a = Vsl.reshape(NCORES, NG, 4, 2, P, D)         # (i, g, j, c, p, d)
        return np.ascontiguousarray(
            a.transpose(0, 1, 4, 2, 3, 5).astype(h))    # (i, g, p, j, c, d)

    ksa = kfat(K_sa)
    ka0 = kfat(K_att[:, :, :D])
    kaf = kfat(K_att[:, :, D:])
    vsa = vfat(V_sa)
    va0 = vfat(V_att[:, :, :D])

    hT = np.ascontiguousarray(
        h_t.reshape(NCORES, BC, 4, P).transpose(0, 3, 2, 1).astype(h))

    maskf = np.where(mask, f(-30000.0), f(0.0)).astype(f)           # (B, NK)
    # mask tiles: rows 32j..32j+16 = maskf[4t+j]; pad cols -30000
    mask2 = np.zeros((NCORES, NG, 4, 32, NP), f)
    mask2[..., :16, :NK] = maskf.reshape(NCORES, NG, 4, 1, NK)
    mask2[..., :16, NK:] = -30000.0

    # constant masks
    ident = np.eye(P, dtype=h)
    ones4 = np.zeros((P, 4), h)
    for j in range(4):
        ones4[32 * j:32 * j + 16, j] = 1.0
    blk4 = np.zeros((P, 4), h)
    for j in range(4):
        blk4[32 * j:32 * j + 16, j] = 1.0
    bm4 = np.zeros((P, D), f)
    for j in range(4):
        for hh in range(H):
            bm4[32 * j + hh, DH * hh:DH * (hh + 1)] = 1.0
    # Qbd mask: for d-chunk c, row p maps to d = 128c + p, head = d // 32
    qm = np.zeros((P, 4, DH), h)
    for c in range(4):
        for p in range(P):
            hh = (c * P + p) // DH
            qm[p, c, hh] = 1.0

    W16 = {n: np.ascontiguousarray(
        W[n].reshape(4, P, D).transpose(1, 0, 2).astype(h)) for n in WNAMES}
    b_dmaj = {n: np.ascontiguousarray(bvec[n].reshape(4, P).T) for n in WNAMES}

    in_maps = []
    for i in range(NCORES):
        m = {
            "hT": hT[i],
            "h_bm": np.ascontiguousarray(h_t[BC * i:BC * (i + 1)]),
            "ksa": ksa[i],
            "vsa": vsa[i],
            "ka0": ka0[i],
            "va0": va0[i],
            "kaf": kaf[i],
            "mask2": mask2[i].reshape(NG, P, NP).astype(np.float16),
            "ident": ident,
            "ones4": ones4,
            "blk4": blk4,
            "bm4": bm4,
            "qm": qm,
        }
        for n in WNAMES:
            m["W_" + n] = W16[n]
            if ub[n]:
                m["b_" + n] = b_dmaj[n]
                if n in BM_BIAS:
                    m["bf_" + n] = bvec[n].reshape(1, D)
        if any(ln_affine):
            m["lnp"] = lnp
        in_maps.append(m)
    return flags, in_maps


def _run(inputs, trace=False):
    flags, in_maps = _prep_inputs(inputs)
    nc = _get_program(flags)
    kwargs = {}
    if trace:
        kwargs = dict(trace=True, trace_cores=[0])
    res = run_bass_kernel_spmd(nc, in_maps, list(range(NCORES)), **kwargs)
    out = np.concatenate([res.results[i]["out"] for i in range(NCORES)], axis=0)
    return np.ascontiguousarray(out.astype(np.float32)), res


def kernel(**inputs):
    return _run(inputs, trace=False)[0]


def kernel_traced(**inputs):
    return _run(inputs, trace=True)


# revision 12
# speedup vs baseline: 1.1031x; 1.1031x over previous
"""Trainium2 Bass kernel for nn_AttentionModel (pointer-network decode step).

Data-parallel over 8 NeuronCores: batch 512 -> 64 samples/core; weights
replicated.  v5: all streamed K/V slabs and weights are fp16 (~85 MB/core of
HBM reads) with host-side layouts interleaved per 4-sample group so every DMA
is one ~1 MB transfer with 8 KB-contiguous per-partition descriptors.  Every
tile pool (SBUF and PSUM) is allocated once at top level and never released
mid-program: pool-release boundaries execute on the sync queue and would park
it — stalling the stream DMAs queued behind them — for the whole inter-phase
compute chain.  With no mid-program releases the DMA queues stream the next
phase's K/V through each phase boundary.  Attention group loops are
software-pipelined (scores of group g overlap softmax+values of group g-1).

Per core the network is:
  self-attn over (K_sa | k_sa) -> LN -> enc attention (masked) -> LN ->
  MLP -> LN -> single-head tanh-clipped pointer scores -> softmax weights.

Layouts (per core, 64 samples):
  - activations kept batch-major f32 [64, 512] (layernorm/residual) and
    d-major fp16 [128, 4, 64] (matmul operands).
  - K streams host-transposed d-major per group [g, 128, 4j, 4c, 256] fp16;
    scores accumulate over 4 d-chunks with keys as the moving dim.  V streams
    key-major [g, 128, 4j, 2c, 512] fp16.
  - per-sample multi-head scores live in PSUM as 16-head bands at 32-row
    offsets (4 samples per [128, n] tile, PE column-group packing); softmax on
    whole tiles.  esc -> PE transpose (fp16) -> value matmuls; the appended
    self-attn token is handled by a per-group rank-4 matmul (E4 transpose of
    the esc extra column x the 4 new-token value rows).
"""

import numpy as np
from contextlib import ExitStack

import concourse.bass as bass
import concourse.tile as tile
from concourse import bacc, mybir
from concourse.bass_utils import run_bass_kernel_spmd

f32 = mybir.dt.float32
f16 = mybir.dt.float16
AF = mybir.ActivationFunctionType
ALU = mybir.AluOpType
AX = mybir.AxisListType

P = 128          # SBUF partitions
NCORES = 8
B = 512          # full batch
BC = B // NCORES # batch per core (64)
D = 512          # model dim
H = 16           # heads
DH = 32          # head dim
NK = 251         # encoder keys (nb_nodes + 1)
NP = 256         # encoder keys padded to 256
T = 256          # self-attn cache length (new token appended on device)
NG = BC // 4     # sample groups of 4 (one [128, n] psum tile each)

WNAMES = ["Wq_sa", "Wk_sa", "Wv_sa", "W0_sa", "Wq_a", "W0_a", "W1", "W2", "Wqf"]
EARLY_W = ["Wq_sa", "Wk_sa", "Wv_sa"]   # needed before layer-1 attention
# weight matmuls whose bias is applied on batch-major [64, 512] rows
BM_BIAS = {"Wv_sa", "W0_sa", "W0_a", "W2"}

_cache = {}


def _expandcol(ap3, n):
    """Replace a trailing size-1 dim with a 0-stride dim of size n."""
    a = [list(x) for x in ap3.ap]
    assert a[-1][1] == 1
    a[-1] = [0, n]
    return bass.AP(tensor=ap3.tensor, offset=ap3.offset, ap=a)


def _bcast_row(ap2d, i, n):
    row = ap2d[i:i + 1, :]
    return bass.AP(tensor=row.tensor, offset=row.offset,
                   ap=[[0, n]] + list(row.ap)[1:])


# ----------------------------------------------------------------------------
# program builder
# ----------------------------------------------------------------------------

def _build_program(flags):
    """flags = (use_bias tuple aligned with WNAMES, ln_affine tuple of 3)."""
    use_bias = dict(zip(WNAMES, flags[0]))
    ln_affine = flags[1]

    nc = bacc.Bacc("TRN2", target_bir_lowering=False, debug=False)

    def din(name, shape, dt=f32):
        return nc.dram_tensor(name, shape, dt, kind="ExternalInput").ap()

    hT_d = din("hT", [P, 4, BC], f16)
    hbm_d = din("h_bm", [BC, D])
    ksa_d = din("ksa", [NG, P, 4, 4, NP], f16)
    vsa_d = din("vsa", [NG, P, 4, 2, D], f16)
    ka0_d = din("ka0", [NG, P, 4, 4, NP], f16)
    va0_d = din("va0", [NG, P, 4, 2, D], f16)
    kaf_d = din("kaf", [NG, P, 4, 4, NP], f16)
    mask2_d = din("mask2", [NG, P, NP], f16)
    W_d = {n: din("W_" + n, [P, 4, D], f16) for n in WNAMES}
    b_d = {n: din("b_" + n, [P, 4]) for n in WNAMES if use_bias[n]}
    bf_d = {n: din("bf_" + n, [1, D]) for n in WNAMES
            if use_bias[n] and n in BM_BIAS}
    if any(ln_affine):
        lnp_d = din("lnp", [6, D])
    ident_d = din("ident", [P, P], f16)
    identF_d = din("identF", [P, P])
    ones4_d = din("ones4", [P, 4], f16)
    blk4_d = din("blk4", [P, 4], f16)
    bm4_d = din("bm4", [P, D])
    qm_d = din("qm", [P, 4, DH], f16)
    vscr_d = nc.dram_tensor("vscr", [BC, D], f16, kind="Internal").ap()

    out_d = nc.dram_tensor("out", [BC, NK], f32, kind="ExternalOutput").ap()

    with tile.TileContext(nc) as tc, ExitStack() as ctx:
        pool = lambda name, bufs, **kw: ctx.enter_context(
            tc.tile_pool(name=name, bufs=bufs, **kw))

        consts = pool("consts", 1)
        acts = pool("acts", 1)
        small = pool("small", 24)
        big_tmp = pool("big_tmp", 2)
        # stream pools shared by all three attention phases; deep enough that
        # DMA keeps prefetching through the serial x-compute at phase edges
        pk = pool("pk", 6)
        pv = pool("pv", 6)
        # attention-side pools
        pqbd = pool("pqbd", 2)
        pesc = pool("pesc", 2)
        pwt = pool("pwt", 2)
        pe4 = pool("pe4", 2)
        pw256 = pool("pw256", 2)
        pex = pool("pex", 2)
        pa4 = pool("pa4", 2)
        # final-phase pools
        pft = pool("pft", 2)
        pfe = pool("pfe", 2)
        pfw = pool("pfw", 2)
        # PSUM: exactly 8 banks (PSUM tiles are bank-aligned per tile):
        # psc 2 + pswt 1 + pspt 2 + pacc 1 + pmj 2.
        psc = pool("psc", 2, space="PSUM")       # scores [P, NP+1] f32
        pswt = pool("pswt", 1, space="PSUM")     # esc/e4 transposes f16
        pspt = pool("pspt", 2, space="PSUM")     # value products [P, D] f32
        pacc = pool("pacc", 1, space="PSUM")     # head fold + batch-major mm
        pmj = pool("pmj", 2, space="PSUM")       # proj outputs + bm transposes

        # ------------------------------------------------------------------
        # constants / early weights (late weights stream during layer 1)
        # ------------------------------------------------------------------
        qm = consts.tile([P, 4, DH], f16, name="qm", tag="qm")
        nc.scalar.dma_start(out=qm, in_=qm_d)
        ident = consts.tile([P, P], f16, name="ident", tag="ident")
        nc.scalar.dma_start(out=ident, in_=ident_d)
        identF = consts.tile([P, P], f32, name="identF", tag="identF")
        nc.scalar.dma_start(out=identF, in_=identF_d)
        ones4 = consts.tile([P, 4], f16, name="ones4", tag="ones4")
        nc.scalar.dma_start(out=ones4, in_=ones4_d)
        blk4 = consts.tile([P, 4], f16, name="blk4", tag="blk4")
        nc.scalar.dma_start(out=blk4, in_=blk4_d)
        bm4 = consts.tile([P, D], f32, name="bm4", tag="bm4")
        nc.scalar.dma_start(out=bm4, in_=bm4_d)
        eps = consts.tile([P, 1], f32, name="eps", tag="eps")
        nc.vector.memset(eps, 1e-5)

        x0T = acts.tile([P, 4, BC], f16, name="x0T", tag="x0T")
        nc.scalar.dma_start(out=x0T, in_=hT_d)
        h_bm = acts.tile([BC, D], f32, name="h_bm", tag="h_bm")
        nc.scalar.dma_start(out=h_bm, in_=hbm_d)

        mask_all = consts.tile([P, NG, NP], f16, name="mask_all", tag="mask_all")
        nc.gpsimd.dma_start(
            out=mask_all, in_=mask2_d.rearrange("g p n -> p g n"))

        Wt, bt, bft = {}, {}, {}
        for n in WNAMES:
            Wt[n] = consts.tile([P, 4, D], f16, name="W_" + n, tag="W_" + n)
            if use_bias[n]:
                bt[n] = consts.tile([P, 4], f32, name="b_" + n, tag="b_" + n)
                if n in BM_BIAS:
                    bft[n] = consts.tile([BC, D], f32, name="bf_" + n,
                                         tag="bf_" + n)

        def load_weight(n, engine):
            engine.dma_start(out=Wt[n], in_=W_d[n])
            if use_bias[n]:
                nc.scalar.dma_start(out=bt[n], in_=b_d[n])
                if n in BM_BIAS:
                    nc.scalar.dma_start(out=bft[n], in_=_bcast_row(bf_d[n], 0, BC))

        for n in EARLY_W:
            load_weight(n, nc.sync)

        lng, lnb = [None] * 3, [None] * 3
        for i in range(3):
            if ln_affine[i]:
                lng[i] = consts.tile([BC, D], f32, name=f"lng{i}", tag=f"lng{i}")
                nc.scalar.dma_start(out=lng[i], in_=_bcast_row(lnp_d, 2 * i, BC))
                lnb[i] = consts.tile([BC, D], f32, name=f"lnb{i}", tag=f"lnb{i}")
                nc.scalar.dma_start(out=lnb[i], in_=_bcast_row(lnp_d, 2 * i + 1, BC))

        # ------------------------------------------------------------------
        # helpers
        # ------------------------------------------------------------------
        def proj_dmajor(dst, wname, src_T, relu=False):
            """dst[:, mc, :] (d-major fp16 [128, 4, 64]) = act(x @ W + b)."""
            for mc in range(4):
                ps = pmj.tile([P, BC], f32, name="ps_pj", tag="ps_pj")
                for kc in range(4):
                    nc.tensor.matmul(
                        ps,
                        lhsT=Wt[wname][:, kc, mc * P:(mc + 1) * P],
                        rhs=src_T[:, kc, :],
                        start=(kc == 0), stop=(kc == 3),
                    )
                bias = bt[wname][:, mc:mc + 1] if use_bias[wname] else 0.0
                func = AF.Relu if relu else AF.Identity
                nc.scalar.activation(dst[:, mc, :], ps, func, bias=bias,
                                     scale=1.0)

        def mm_batchmajor(ps, src_T, wname):
            """ps [64, 512] f32 = x @ W   (lhsT = x^T chunks, W moving)."""
            for kc in range(4):
                nc.tensor.matmul(
                    ps,
                    lhsT=src_T[:, kc, :],
                    rhs=Wt[wname][:, kc, :],
                    start=(kc == 0), stop=(kc == 3),
                )

        def to_dmajor(dst_T, src_bm):
            """[64, 512] f32 batch-major -> d-major fp16 [128, 4, 64]."""
            for c in range(4):
                ps = pmj.tile([P, BC], f32, name="ps_pj", tag="ps_pj")
                nc.tensor.transpose(ps, src_bm[:, c * P:(c + 1) * P],
                                    identF[0:BC, 0:BC])
                nc.scalar.copy(dst_T[:, c, :], ps)

        def layer_norm(dst_bm, t_bm, idx):
            stats = small.tile([BC, 6], f32, name="stats", tag="stats")
            nc.vector.bn_stats(stats, t_bm)
            mv = small.tile([BC, 2], f32, name="mv", tag="mv")
            nc.vector.bn_aggr(mv, stats)
            sd = small.tile([BC, 1], f32, name="sd", tag="sd")
            nc.scalar.activation(sd, mv[:, 1:2], AF.Sqrt, bias=eps[0:BC],
                                 scale=1.0)
            rstd = small.tile([BC, 1], f32, name="rstd", tag="rstd")
            nc.vector.reciprocal(rstd, sd)
            nmr = small.tile([BC, 1], f32, name="nmr", tag="nmr")
            nc.vector.scalar_tensor_tensor(out=nmr, in0=mv[:, 0:1], scalar=-1.0,
                                           in1=rstd, op0=ALU.mult, op1=ALU.mult)
            if ln_affine[idx]:
                xn = big_tmp.tile([BC, D], f32, name="xn", tag="xn")
                nc.scalar.activation(xn, t_bm, AF.Identity, bias=nmr, scale=rstd)
                nc.vector.tensor_mul(xn, xn, lng[idx])
                nc.vector.tensor_add(dst_bm, xn, lnb[idx])
            else:
                nc.scalar.activation(dst_bm, t_bm, AF.Identity, bias=nmr,
                                     scale=rstd)

        def residual_ln(dst_bm, dst_T, src_T, wname, x_prev_bm, idx):
            """dst = LN(x_prev + src @ W + b); also d-major fp16 dst_T."""
            ps = pacc.tile([BC, D], f32, name="ps_acc", tag="ps_acc")
            mm_batchmajor(ps, src_T, wname)
            t_bm = big_tmp.tile([BC, D], f32, name="t_bm", tag="t_bm")
            nc.vector.tensor_add(t_bm, ps, x_prev_bm)
            if use_bias[wname]:
                nc.vector.tensor_add(t_bm, t_bm, bft[wname])
            layer_norm(dst_bm, t_bm, idx)
            to_dmajor(dst_T, dst_bm)

        # ------------------------------------------------------------------
        # projections from x0 = h_t
        # ------------------------------------------------------------------
        q_saT = acts.tile([P, 4, BC], f16, name="q_saT", tag="q_saT")
        proj_dmajor(q_saT, "Wq_sa", x0T)
        k_saT = acts.tile([P, 4, BC], f16, name="k_saT", tag="k_saT")
        proj_dmajor(k_saT, "Wk_sa", x0T)

        # v = x0 @ Wv_sa, fp16 batch-major, then regrouped [4, 16, 512]
        # (vr4[j, g, :] = v[4g + j]) for the per-group rank-4 extra-value mm.
        v16 = acts.tile([BC, D], f16, name="v16", tag="v16")
        psv = pacc.tile([BC, D], f32, name="ps_acc", tag="ps_acc")
        mm_batchmajor(psv, x0T, "Wv_sa")
        if use_bias["Wv_sa"]:
            vtmp = big_tmp.tile([BC, D], f32, name="vtmp", tag="vtmp")
            nc.vector.tensor_add(vtmp, psv, bft["Wv_sa"])
            nc.vector.tensor_copy(v16, vtmp)
        else:
            nc.vector.tensor_copy(v16, psv)
        vr4 = acts.tile([4, NG, D], f16, name="vr4", tag="vr4")
        nc.scalar.dma_start(out=vscr_d, in_=v16)
        vsr = vscr_d.rearrange("(g j) d -> j g d", j=4)
        for j in range(4):
            nc.scalar.dma_start(out=vr4[j:j + 1, :, :], in_=vsr[j:j + 1])

        # ------------------------------------------------------------------
        # attention layers (software-pipelined over 16 groups of 4 samples)
        # ------------------------------------------------------------------
        def attention(qT, kstream_d, vstream_d, attn16, extra, masked,
                      after_scores0=None):
            """attn16 [64, 512] fp16 <- per-sample MHA.

            extra: include the appended self-attn token (k from k_saT, v from
            vr4).  masked: add the -30000 encoder mask tiles before softmax.
            after_scores0: callback emitted after group 0's scores (used to
            stream the late weights without delaying the first attention DMA).
            """
            ncol = NP + 1 if extra else NP

            def stage_scores(g):
                kt = pk.tile([P, 4, 4, NP], f16, name="kt", tag="kt")
                nc.sync.dma_start(out=kt, in_=kstream_d[g])
                vt = pv.tile([P, 4, 2, D], f16, name="vt", tag="vt")
                nc.sync.dma_start(out=vt, in_=vstream_d[g])
                qbd = pqbd.tile([P, 4, 4, DH], f16, name="qbd", tag="qbd")
                for j in range(4):
                    b = 4 * g + j
                    nc.vector.tensor_mul(
                        qbd[:, j, :, :], qm,
                        _expandcol(qT[:, :, b:b + 1], DH))
                ps_sc = psc.tile([P, NP + 1], f32, name="ps_sc", tag="ps_sc")
                for j in range(4):
                    b = 4 * g + j
                    for c in range(4):
                        nc.tensor.matmul(
                            ps_sc[32 * j:32 * j + 32, 0:NP],
                            lhsT=qbd[:, j, c, :],
                            rhs=kt[:, j, c, :],
                            start=(c == 0), stop=(False if extra else c == 3),
                            tile_position=(0, 32 * j))
                        if extra:
                            nc.tensor.matmul(
                                ps_sc[32 * j:32 * j + 32, NP:NP + 1],
                                lhsT=qbd[:, j, c, :],
                                rhs=k_saT[:, c, b:b + 1],
                                start=False, stop=(c == 3),
                                tile_position=(0, 32 * j))
                return ps_sc, vt

            def stage_rest(g, ps_sc, vt):
                scv = ps_sc[:, 0:ncol]
                if masked:
                    nc.vector.tensor_add(ps_sc[:, 0:NP], ps_sc[:, 0:NP],
                                         mask_all[:, g, :])
                negmax = small.tile([P, 1], f32, name="negmax", tag="negmax")
                nc.vector.tensor_reduce(negmax, scv, axis=AX.X,
                                        op=ALU.max, negate=True)
                esc = pesc.tile([P, NP + 1], f16, name="esc", tag="esc")
                sumexp = small.tile([P, 1], f32, name="sumexp", tag="sumexp")
                nc.scalar.activation(esc[:, 0:ncol], scv, AF.Exp, bias=negmax,
                                     scale=1.0, accum_out=sumexp)
                recip = small.tile([P, 1], f32, name="recip", tag="recip")
                nc.vector.reciprocal(recip, sumexp)

                ps_wt = pswt.tile([P, 3, P], f16, name="ps_wt", tag="ps_wt")
                for c in range(2):
                    nc.tensor.transpose(ps_wt[:, c, :],
                                        esc[:, c * P:(c + 1) * P], ident)
                wt = pwt.tile([P, 2, P], f16, name="wt", tag="wt")
                for c in range(2):
                    nc.scalar.copy(wt[:, c, :], ps_wt[:, c, :])

                if extra:
                    e4 = pe4.tile([P, 4], f16, name="e4", tag="e4")
                    nc.vector.tensor_mul(
                        e4, blk4, _expandcol(esc[:, NP:NP + 1], 4))
                    nc.tensor.transpose(ps_wt[0:4, 2, :], e4, ident)
                    w256 = pw256.tile([4, P], f16, name="w256", tag="w256")
                    nc.scalar.copy(w256, ps_wt[0:4, 2, :])

                ps_pt = pspt.tile([P, D], f32, name="ps_pt", tag="ps_pt")
                for j in range(4):
                    for kc in range(2):
                        nc.tensor.matmul(
                            ps_pt[32 * j:32 * j + 32, :],
                            lhsT=wt[:, kc, 32 * j:32 * j + 32],
                            rhs=vt[:, j, kc, :],
                            start=(kc == 0), stop=(kc == 1),
                            tile_position=(0, 32 * j))
                if extra:
                    nc.tensor.matmul(
                        ps_pt, lhsT=w256, rhs=vr4[:, g, :],
                        start=False, stop=True, skip_group_check=True)
                ex = pex.tile([P, D], f16, name="ex", tag="ex")
                nc.vector.scalar_tensor_tensor(
                    out=ex, in0=ps_pt, scalar=recip, in1=bm4,
                    op0=ALU.mult, op1=ALU.mult)
                ps_acc = pacc.tile([BC, D], f32, name="ps_acc", tag="ps_acc")
                nc.tensor.matmul(ps_acc[0:4, :], lhsT=ones4, rhs=ex,
                                 start=True, stop=True)
                a4 = pa4.tile([4, D], f32, name="a4", tag="a4")
                nc.vector.tensor_copy(a4, ps_acc[0:4, :])
                nc.scalar.dma_start(out=attn16[4 * g:4 * g + 4, :], in_=a4)

            state = {}
            for g in range(NG + 1):
                if g < NG:
                    state[g] = stage_scores(g)
                    if g == 0 and after_scores0 is not None:
                        after_scores0()
                if g > 0:
                    stage_rest(g - 1, *state.pop(g - 1))

        # ------------------------------------------------------------------
        # layer 1: self-attention over (K_sa | k_sa)
        # ------------------------------------------------------------------
        def load_late_weights():
            for n in WNAMES:
                if n not in EARLY_W:
                    load_weight(n, nc.gpsimd)

        attn1_bm = acts.tile([BC, D], f32, name="attn1_bm", tag="attn1_bm")
        attention(q_saT, ksa_d, vsa_d, attn1_bm, extra=True, masked=False,
                  after_scores0=load_late_weights)

        attn1T = acts.tile([P, 4, BC], f16, name="attn1T", tag="attn1T")
        to_dmajor(attn1T, attn1_bm)
        x1_bm = acts.tile([BC, D], f32, name="x1_bm", tag="x1_bm")
        x1T = acts.tile([P, 4, BC], f16, name="x1T", tag="x1T")
        residual_ln(x1_bm, x1T, attn1T, "W0_sa", h_bm, 0)

        # ------------------------------------------------------------------
        # layer 2: encoder attention (masked, padded keys)
        # ------------------------------------------------------------------
        q_aT = acts.tile([P, 4, BC], f16, name="q_aT", tag="q_aT")
        proj_dmajor(q_aT, "Wq_a", x1T)
        attn2_bm = acts.tile([BC, D], f32, name="attn2_bm", tag="attn2_bm")
        attention(q_aT, ka0_d, va0_d, attn2_bm, extra=False, masked=True)

        attn2T = acts.tile([P, 4, BC], f16, name="attn2T", tag="attn2T")
        to_dmajor(attn2T, attn2_bm)
        x2_bm = acts.tile([BC, D], f32, name="x2_bm", tag="x2_bm")
        x2T = acts.tile([P, 4, BC], f16, name="x2T", tag="x2T")
        residual_ln(x2_bm, x2T, attn2T, "W0_a", x1_bm, 1)

        # ------------------------------------------------------------------
        # MLP
        # ------------------------------------------------------------------
        h1T = acts.tile([P, 4, BC], f16, name="h1T", tag="h1T")
        proj_dmajor(h1T, "W1", x2T, relu=True)
        x3_bm = acts.tile([BC, D], f32, name="x3_bm", tag="x3_bm")
        x3T = acts.tile([P, 4, BC], f16, name="x3T", tag="x3T")
        residual_ln(x3_bm, x3T, h1T, "W2", x2_bm, 2)

        qfT = acts.tile([P, 4, BC], f16, name="qfT", tag="qfT")
        proj_dmajor(qfT, "Wqf", x3T)

        # ------------------------------------------------------------------
        # final pointer scores: w = softmax(10*tanh(qf.K/sqrt(D)) + mask)
        # ------------------------------------------------------------------
        def f_scores(g):
            kt = pk.tile([P, 4, 4, NP], f16, name="kt", tag="kt")
            nc.sync.dma_start(out=kt, in_=kaf_d[g])
            ps_f = psc.tile([P, NP + 1], f32, name="ps_sc", tag="ps_sc")
            for j in range(4):
                b = 4 * g + j
                for c in range(4):
                    nc.tensor.matmul(
                        ps_f[32 * j:32 * j + 1, 0:NP],
                        lhsT=qfT[:, c, b:b + 1],
                        rhs=kt[:, j, c, :],
                        start=(c == 0), stop=(c == 3),
                        tile_position=(0, 32 * j))
            return (ps_f,)

        def f_rest(g, ps_f):
            t1 = pft.tile([P, NP], f32, name="t1", tag="t1")
            nc.scalar.activation(t1, ps_f[:, 0:NP], AF.Tanh,
                                 scale=float(D) ** -0.5)
            t2 = pft.tile([P, NP], f32, name="t2", tag="t2")
            nc.vector.scalar_tensor_tensor(out=t2, in0=t1, scalar=10.0,
                                           in1=mask_all[:, g, :],
                                           op0=ALU.mult, op1=ALU.add)
            e = pfe.tile([P, NP], f32, name="e", tag="e")
            sumexp = small.tile([P, 1], f32, name="fsum", tag="fsum")
            nc.scalar.activation(e, t2, AF.Exp, accum_out=sumexp)
            recip = small.tile([P, 1], f32, name="frec", tag="frec")
            nc.vector.reciprocal(recip, sumexp)
            wf = pfw.tile([P, NK], f32, name="wf", tag="wf")
            nc.vector.tensor_scalar_mul(wf, e[:, 0:NK], recip)
            nc.scalar.dma_start(
                out=out_d[4 * g:4 * g + 4, :],
                in_=wf.rearrange("(a b) n -> a b n", b=32)[:, 0, :])

        state = {}
        for g in range(NG + 1):
            if g < NG:
                state[g] = f_scores(g)
            if g > 0:
                f_rest(g - 1, *state.pop(g - 1))

    nc.compile()
    return nc


# ----------------------------------------------------------------------------
# host side
# ----------------------------------------------------------------------------

def _get_program(flags):
    if flags not in _cache:
        _cache[flags] = _build_program(flags)
    return _cache[flags]


def _prep_inputs(inputs):
    """Host-side sharding + layout prep; returns (flags, per-core inputs)."""
    f = np.float32
    h = np.float16
    h_t = np.asarray(inputs["h_t"], f)
    K_att = np.asarray(inputs["K_att"], f)
    V_att = np.asarray(inputs["V_att"], f)
    K_sa = np.asarray(inputs["K_sa"], f)
    V_sa = np.asarray(inputs["V_sa"], f)
    mask = np.asarray(inputs["mask"])

    sc = np.float32(DH ** -0.5)
    W = {n: np.asarray(inputs[n], f) for n in WNAMES}
    W["Wq_sa"] = W["Wq_sa"] * sc
    W["Wq_a"] = W["Wq_a"] * sc
    bias_src = {"Wq_sa": "bq_sa", "Wk_sa": "bk_sa", "Wv_sa": "bv_sa",
                "W0_sa": "b0_sa", "Wq_a": "bq_a", "W0_a": "b0_a",
                "W1": "b1", "W2": "b2", "Wqf": "bqf"}
    bvec = {n: np.asarray(inputs[bias_src[n]], f).copy() for n in WNAMES}
    bvec["Wq_sa"] *= sc
    bvec["Wq_a"] *= sc
    use_bias = tuple(bool(np.any(bvec[n])) for n in WNAMES)
    ub = dict(zip(WNAMES, use_bias))

    lnp = np.stack([np.asarray(inputs[k], f) for k in
                    ["ln1_g", "ln1_b", "ln2_g", "ln2_b", "ln3_g", "ln3_b"]])
    ln_affine = tuple(
        bool(np.any(lnp[2 * i] != 1.0) or np.any(lnp[2 * i + 1] != 0.0))
        for i in range(3))
    flags = (use_bias, ln_affine)

    # d-major group-interleaved K streams: [NCORES, NG, 128, 4j, 4c, 256] fp16
    def kfat(Ksl):
        kT = Ksl.transpose(0, 2, 1)                     # (B, D, n)
        if kT.shape[2] != NP:
            kp = np.zeros((B, D, NP), f)
            kp[:, :, :kT.shape[2]] = kT
            kT = kp
        a = kT.reshape(NCORES, NG, 4, 4, P, NP)         # (i, g, j, c, p, k)
        return np.ascontiguousarray(
            a.transpose(0, 1, 4, 2, 3, 5).astype(h))    # (i, g, p, j, c, k)

    # key-major group-interleaved V streams: [NCORES, NG, 128, 4j, 2c, 512]
    def vfat(Vsl):
        if Vsl.shape[1] != NP:
            vp = np.zeros((B, NP, D), f)
            vp[:, :Vsl.shape[1], :] = Vsl
            Vsl = vp
        a = Vsl.reshape(NCORES, NG, 4, 2, P, D)         # (i, g, j, c, p, d)
        return np.ascontiguousarray(
            a.transpose(0, 1, 4, 2, 3, 5).astype(h))    # (i, g, p, j, c, d)

    ksa = kfat(K_sa)
    ka0 = kfat(K_att[:, :, :D])
    kaf = kfat(K_att[:, :, D:])
    vsa = vfat(V_sa)
    va0 = vfat(V_att[:, :, :D])

    hT = np.ascontiguousarray(
        h_t.reshape(NCORES, BC, 4, P).transpose(0, 3, 2, 1).astype(h))

    maskf = np.where(mask, f(-30000.0), f(0.0)).astype(f)           # (B, NK)
    # mask tiles: rows 32j..32j+16 = maskf[4t+j]; pad cols -30000
    mask2 = np.zeros((NCORES, NG, 4, 32, NP), f)
    mask2[..., :16, :NK] = maskf.reshape(NCORES, NG, 4, 1, NK)
    mask2[..., :16, NK:] = -30000.0

    # constant masks
    ident = np.eye(P, dtype=h)
    identF = np.eye(P, dtype=f)
    ones4 = np.zeros((P, 4), h)
    for j in range(4):
        ones4[32 * j:32 * j + 16, j] = 1.0
    blk4 = np.zeros((P, 4), h)
    for j in range(4):
        blk4[32 * j:32 * j + 16, j] = 1.0
    bm4 = np.zeros((P, D), f)
    for j in range(4):
        for hh in range(H):
            bm4[32 * j + hh, DH * hh:DH * (hh + 1)] = 1.0
    # Qbd mask: for d-chunk c, row p maps to d = 128c + p, head = d // 32
    qm = np.zeros((P, 4, DH), h)
    for c in range(4):
        for p in range(P):
            hh = (c * P + p) // DH
            qm[p, c, hh] = 1.0

    W16 = {n: np.ascontiguousarray(
        W[n].reshape(4, P, D).transpose(1, 0, 2).astype(h)) for n in WNAMES}
    b_dmaj = {n: np.ascontiguousarray(bvec[n].reshape(4, P).T) for n in WNAMES}

    in_maps = []
    for i in range(NCORES):
        m = {
            "hT": hT[i],
            "h_bm": np.ascontiguousarray(h_t[BC * i:BC * (i + 1)]),
            "ksa": ksa[i],
            "vsa": vsa[i],
            "ka0": ka0[i],
            "va0": va0[i],
            "kaf": kaf[i],
            "mask2": mask2[i].reshape(NG, P, NP).astype(np.float16),
            "ident": ident,
            "identF": identF,
            "ones4": ones4,
            "blk4": blk4,
            "bm4": bm4,
            "qm": qm,
        }
        for n in WNAMES:
            m["W_" + n] = W16[n]
            if ub[n]:
                m["b_" + n] = b_dmaj[n]
                if n in BM_BIAS:
                    m["bf_" + n] = bvec[n].reshape(1, D)
        if any(ln_affine):
            m["lnp"] = lnp
        in_maps.append(m)
    return flags, in_maps


def _run(inputs, trace=False):
    flags, in_maps = _prep_inputs(inputs)
    nc = _get_program(flags)
    kwargs = {}
    if trace:
        kwargs = dict(trace=True, trace_cores=[0])
    res = run_bass_kernel_spmd(nc, in_maps, list(range(NCORES)), **kwargs)
    out = np.concatenate([res.results[i]["out"] for i in range(NCORES)], axis=0)
    return np.ascontiguousarray(out.astype(np.float32)), res


def kernel(**inputs):
    return _run(inputs, trace=False)[0]


def kernel_traced(**inputs):
    return _run(inputs, trace=True)
